# revision 2
# baseline (speedup 1.0000x reference)
"""Discrete Hawkes conditional-intensity kernel for 8 Trainium2 NeuronCores.

Math
----
Reference computes, per query i with (t, s) = (t_i, s_i):

    lam_i = clip(mu[s] + alpha[s, s] * b * F[t, s], 1e-5)
    F[t, s] = sum_{tp < t} obs[tp, s] * exp(-b * (t - tp))

With t = j*128 + p (j time-block of 128) and b >= 0.1 (spec: beta =
uniform+0.1), the carry from blocks >= 2 back is attenuated by
exp(-256 b) <= e^{-25.6} -- far below the 2e-2 tolerance -- so F
collapses to the within-block term plus ONE neighbour-block term:

    F[j*128+p, s] ~= sum_{q<p} obs[j*128+q, s] e^{-b(p-q)}         (MM1)
                   + sum_{q}   obs[(j-1)*128+q, s] e^{-b(128+p-q)} (MM2)

Both are 128-deep matmuls over the shared rhs obs_f1[q, (j, s)] =
obs * alpha_diag * b; MM2 reads the rhs shifted one block (32 cols)
left via a zero block prepended for j = 0.  The decay matrices come
straight from two gpsimd iotas (q-p and q-p-128, sign pre-folded) fed
to Exp with scale=beta -- nothing but the two activations depends on
beta.  The mu add and the clip fuse into the single PSUM->SBUF pass
(scalar_tensor_tensor: out = max(psum, 1e-5) + mu).

Sharding: by SPACE.  Core c owns s in [32c, 32c+32) and builds its
[128, 32*32] slice of the G table; the host picks each query's cell
out[p, u] (p = t mod 128, u = (t div 128)*32 + s_rel) while
un-sharding.  No gathers, no collectives, no cross-engine carry chain.
"""

import os
import sys

import numpy as np

_REPO_CANDIDATES = ("/opt/trn_rl_repo", os.path.expanduser("~/.axon_site/_ro/trn_rl_repo"))
for _p in _REPO_CANDIDATES:
    if os.path.isdir(_p) and _p not in sys.path:
        sys.path.append(_p)

import concourse.bass as bass
import concourse.tile as tile
from concourse import bacc, mybir
from concourse.bass_utils import run_bass_kernel_spmd

# Problem constants (hardcoded per spec).
N_TIME = 4096
N_SPACE = 256
BATCH = 65536
N_CORES = 8
LAM_MIN = 1e-5

P = 128                 # partitions / time-block size
J = N_TIME // P         # 32 time blocks
S = N_SPACE // N_CORES  # 32 space columns per core
HALF = J * S // 2       # 512 free elems per PSUM bank

f32 = mybir.dt.float32
bf16 = mybir.dt.bfloat16
Alu = mybir.AluOpType
Act = mybir.ActivationFunctionType


def build_nc():
    nc = bacc.Bacc("TRN2", target_bir_lowering=False, debug=False)

    obs1_h = nc.dram_tensor("obs1", [P, J * S], bf16, kind="ExternalInput")
    parbeta_h = nc.dram_tensor("parbeta", [P, 1], f32, kind="ExternalInput")
    # [adiag | mu] host-replicated down all 128 partitions
    padm_h = nc.dram_tensor("padm", [P, 2 * S], bf16, kind="ExternalInput")
    out_h = nc.dram_tensor("out", [P * J * S], bf16, kind="ExternalOutput")

    from contextlib import ExitStack

    with tile.TileContext(nc) as tc, ExitStack() as ctx:
        sb = ctx.enter_context(tc.tile_pool(name="sb", bufs=1))
        ps = ctx.enter_context(tc.tile_pool(name="ps", bufs=2, space="PSUM"))

        # ---- input loads (HWDGE only; tiny heads first per queue) --------
        parbeta = sb.tile([P, 1], f32)
        nc.sync.dma_start(parbeta[:], parbeta_h.ap())
        padm = sb.tile([P, 2 * S], bf16)
        nc.scalar.dma_start(padm[:], padm_h.ap())
        obs1 = sb.tile([P, J * S], bf16)
        nc.sync.dma_start(obs1[:, 0:HALF],
                          bass.AP(obs1_h, 0, [[J * S, P], [1, HALF]]))
        nc.scalar.dma_start(obs1[:, HALF:2 * HALF],
                            bass.AP(obs1_h, HALF, [[J * S, P], [1, HALF]]))

        # ---- decay matrices: iotas carry the sign, Exp scales by beta ----
        # xd2[q,p] = q - p ; xdm128[q,p] = q - p - 128  (both input-free)
        xd2 = sb.tile([P, P], f32)
        nc.gpsimd.iota(xd2[:], [[-1, P]], base=0, channel_multiplier=1,
                       allow_small_or_imprecise_dtypes=True)
        xdm2 = sb.tile([P, P], f32)
        nc.gpsimd.affine_select(xdm2[:], xd2[:], [[1, P]], Alu.is_gt, -16384.0,
                                base=0, channel_multiplier=-1)  # keep p - q > 0
        xdm128 = sb.tile([P, P], f32)
        nc.gpsimd.iota(xdm128[:], [[-1, P]], base=-P, channel_multiplier=1,
                       allow_small_or_imprecise_dtypes=True)

        # ldtb[q,p] = exp(-b(p-q)) for q<p else 0; ld2[q,p] = exp(-b(128+p-q))
        ldtb = sb.tile([P, P], bf16)
        nc.scalar.activation(ldtb[:], xdm2[:], Act.Exp, scale=parbeta[:])
        ld2 = sb.tile([P, P], bf16)
        nc.scalar.activation(ld2[:], xdm128[:], Act.Exp, scale=parbeta[:])

        # ---- obs scale: obs_f1 = obs * (adiag * b), bf16 ----------------
        asbb = sb.tile([P, S], bf16)
        nc.vector.tensor_scalar(out=asbb[:], in0=padm[:, 0:S],
                                scalar1=parbeta[:], scalar2=None, op0=Alu.mult)

        # obs_f1x: one zero block, then the J obs blocks -- MM2 reads the
        # same column window as MM1 on this shifted view.
        obs_f1x = sb.tile([P, S + J * S], bf16)
        nc.gpsimd.memset(obs_f1x[:, 0:S], 0.0)
        obs_f1 = obs_f1x[:, S:S + J * S]
        for h in range(2):
            nc.vector.tensor_tensor(
                out=obs_f1[:, h * HALF:(h + 1) * HALF].rearrange(
                    "p (j s) -> p j s", s=S),
                in0=obs1[:, h * HALF:(h + 1) * HALF].rearrange(
                    "p (j s) -> p j s", s=S),
                in1=asbb[:].unsqueeze(1).broadcast_to((P, J // 2, S)),
                op=Alu.mult)

        # ---- G = MM1 + MM2, fused clip+mu on the PSUM->SBUF pass --------
        g_sb = sb.tile([P, J * S], bf16)
        for h in range(2):
            pch = ps.tile([P, HALF], f32)
            nc.tensor.matmul(pch[:], lhsT=ldtb[:],
                             rhs=obs_f1[:, h * HALF:(h + 1) * HALF],
                             start=True, stop=False)
            nc.tensor.matmul(pch[:], lhsT=ld2[:],
                             rhs=obs_f1x[:, h * HALF:(h + 1) * HALF],
                             start=False, stop=True, skip_group_check=True)
            # (gpsimd cannot read PSUM on TRN2 -- both passes on DVE)
            nc.vector.scalar_tensor_tensor(
                out=g_sb[:, h * HALF:(h + 1) * HALF].rearrange(
                    "p (j s) -> p j s", s=S),
                in0=pch[:].rearrange("p (j s) -> p j s", s=S),
                scalar=float(LAM_MIN),
                in1=padm[:, S:2 * S].unsqueeze(1).broadcast_to((P, J // 2, S)),
                op0=Alu.max, op1=Alu.add)
            (nc.sync if h == 0 else nc.scalar).dma_start(
                bass.AP(out_h, h * HALF, [[J * S, P], [1, HALF]]),
                g_sb[:, h * HALF:(h + 1) * HALF])

    nc.compile()
    return nc


_NC_CACHE = None


def _get_nc():
    global _NC_CACHE
    if _NC_CACHE is None:
        _NC_CACHE = build_nc()
    return _NC_CACHE


def _flat_positions(tc_, sc_):
    """Query (t, s) is table cell [p = t mod 128, u = (t div 128)*S + s]
    of the dumped [128, J*S] slice."""
    return (tc_ % P).astype(np.int64) * (J * S) + (tc_ >> 7) * S + sc_


def _make_in_maps(t, s, obs, mu, alpha, beta):
    """Shard by space: core c gets s in [S*c, S*(c+1)).  Returns
    (in_maps, perms) where perms[c] = (flat_out_pos, global_orig_pos)."""
    import ml_dtypes

    t = np.ascontiguousarray(np.asarray(t, dtype=np.int32))
    s = np.ascontiguousarray(np.asarray(s, dtype=np.int32))
    obs = np.ascontiguousarray(np.asarray(obs, dtype=np.int32))
    mu = np.ascontiguousarray(np.asarray(mu, dtype=np.float32))
    alpha = np.asarray(alpha, dtype=np.float32)
    beta = np.ascontiguousarray(np.asarray(beta, dtype=np.float32))
    adiag = np.ascontiguousarray(np.diagonal(alpha)).astype(np.float32)

    parbeta = np.full((P, 1), beta[0], np.float32)

    in_maps, perms = [], []
    for c in range(N_CORES):
        m = (s >> 5) == c
        orig_global = np.nonzero(m)[0]
        flat_pos = _flat_positions(t[m], s[m] & (S - 1))

        o3 = obs[:, S * c:S * (c + 1)].reshape(J, P, S)
        obs1 = np.ascontiguousarray(o3.transpose(1, 0, 2)).reshape(P, J * S)
        padm = np.empty((P, 2 * S), np.float32)
        padm[:, 0:S] = adiag[S * c:S * (c + 1)]
        padm[:, S:2 * S] = mu[S * c:S * (c + 1)]
        in_maps.append({
            "obs1": obs1.astype(ml_dtypes.bfloat16),
            "parbeta": parbeta,
            "padm": padm.astype(ml_dtypes.bfloat16),
        })
        perms.append((flat_pos, orig_global))
    return in_maps, perms


def kernel(t, s, obs, mu, alpha, beta, **_unused):
    nc = _get_nc()
    in_maps, perms = _make_in_maps(t, s, obs, mu, alpha, beta)
    res = run_bass_kernel_spmd(nc, in_maps, core_ids=list(range(N_CORES)))
    out = np.empty(BATCH, np.float32)
    for c in range(N_CORES):
        dev = res.results[c]["out"].reshape(-1)   # [P*J*S]
        out[perms[c][1]] = dev[perms[c][0]]
    return out


if __name__ == "__main__":
    # quick self-check against a numpy re-implementation on random data
    rng = np.random.default_rng(0)
    t = rng.integers(0, N_TIME, BATCH).astype(np.int32)
    s = rng.integers(0, N_SPACE, BATCH).astype(np.int32)
    obs = rng.integers(0, 10, (N_TIME, N_SPACE)).astype(np.int32)
    mu = rng.random(N_SPACE, dtype=np.float32)
    alpha = rng.random((N_SPACE, N_SPACE), dtype=np.float32)
    beta = (rng.random(1, dtype=np.float32) + 0.1).astype(np.float32)

    got = kernel(t=t, s=s, obs=obs, mu=mu, alpha=alpha, beta=beta)

    b = float(beta[0])
    e = np.exp(-b)
    F = np.zeros((N_TIME, N_SPACE), np.float64)
    for tt in range(1, N_TIME):
        F[tt] = e * (F[tt - 1] + obs[tt - 1])
    G = np.clip(mu[None, :] + np.diag(alpha)[None, :] * b * F, LAM_MIN, None)
    want = G[t, s].astype(np.float32)
    err = np.abs(got - want) / np.maximum(np.abs(want), 1e-6)
    print("max rel err:", err.max(), "mean:", err.mean())


# revision 4
# speedup vs baseline: 1.0133x; 1.0133x over previous
"""Discrete Hawkes conditional-intensity kernel for 8 Trainium2 NeuronCores.

Math
----
Reference computes, per query i with (t, s) = (t_i, s_i):

    lam_i = clip(mu[s] + alpha[s, s] * b * F[t, s], 1e-5)
    F[t, s] = sum_{tp < t} obs[tp, s] * exp(-b * (t - tp))

With t = j*128 + p (j time-block of 128) and b >= 0.1 (spec: beta =
uniform+0.1), the carry from blocks >= 2 back is attenuated by
exp(-256 b) <= e^{-25.6} -- far below the 2e-2 tolerance -- so F
collapses to the within-block term plus ONE neighbour-block term:

    F[j*128+p, s] ~= sum_{q<p} obs[j*128+q, s] e^{-b(p-q)}         (MM1)
                   + sum_{q}   obs[(j-1)*128+q, s] e^{-b(128+p-q)} (MM2)

Both are 128-deep matmuls over the shared rhs obs_f1[q, (j, s)] =
obs * alpha_diag * b; MM2 reads the rhs shifted one block (32 cols)
left via a zero block prepended for j = 0.  The decay matrices come
straight from two gpsimd iotas (q-p and q-p-128, sign pre-folded) fed
to Exp with scale=beta -- nothing but the two activations depends on
beta.  The mu add and the clip fuse into the single PSUM->SBUF pass
(scalar_tensor_tensor: out = max(psum, 1e-5) + mu).

Sharding: by SPACE.  Core c owns s in [32c, 32c+32) and builds its
[128, 32*32] slice of the G table; the host picks each query's cell
out[p, u] (p = t mod 128, u = (t div 128)*32 + s_rel) while
un-sharding.  No gathers, no collectives, no cross-engine carry chain.
"""

import os
import sys

import numpy as np

_REPO_CANDIDATES = ("/opt/trn_rl_repo", os.path.expanduser("~/.axon_site/_ro/trn_rl_repo"))
for _p in _REPO_CANDIDATES:
    if os.path.isdir(_p) and _p not in sys.path:
        sys.path.append(_p)

import concourse.bass as bass
import concourse.tile as tile
from concourse import bacc, mybir
from concourse.bass_utils import run_bass_kernel_spmd

# Problem constants (hardcoded per spec).
N_TIME = 4096
N_SPACE = 256
BATCH = 65536
N_CORES = 8
LAM_MIN = 1e-5

P = 128                 # partitions / time-block size
J = N_TIME // P         # 32 time blocks
S = N_SPACE // N_CORES  # 32 space columns per core
HALF = J * S // 2       # 512 free elems per PSUM bank

f32 = mybir.dt.float32
bf16 = mybir.dt.bfloat16
i8 = mybir.dt.int8
Alu = mybir.AluOpType
Act = mybir.ActivationFunctionType


def build_nc():
    nc = bacc.Bacc("TRN2", target_bir_lowering=False, debug=False)

    obs1_h = nc.dram_tensor("obs1", [P, J * S], i8, kind="ExternalInput")
    parbeta_h = nc.dram_tensor("parbeta", [P, 1], f32, kind="ExternalInput")
    # [adiag | mu] host-replicated down all 128 partitions
    padm_h = nc.dram_tensor("padm", [P, 2 * S], bf16, kind="ExternalInput")
    out_h = nc.dram_tensor("out", [P * J * S], bf16, kind="ExternalOutput")

    from contextlib import ExitStack

    with tile.TileContext(nc) as tc, ExitStack() as ctx:
        sb = ctx.enter_context(tc.tile_pool(name="sb", bufs=1))
        ps = ctx.enter_context(tc.tile_pool(name="ps", bufs=2, space="PSUM"))

        # ---- input loads (HWDGE only; tiny heads first per queue) --------
        parbeta = sb.tile([P, 1], f32)
        nc.sync.dma_start(parbeta[:], parbeta_h.ap())
        padm = sb.tile([P, 2 * S], bf16)
        nc.scalar.dma_start(padm[:], padm_h.ap())
        obs1 = sb.tile([P, J * S], i8)
        nc.sync.dma_start(obs1[:, 0:HALF],
                          bass.AP(obs1_h, 0, [[J * S, P], [1, HALF]]))
        nc.scalar.dma_start(obs1[:, HALF:2 * HALF],
                            bass.AP(obs1_h, HALF, [[J * S, P], [1, HALF]]))

        # ---- decay matrices: iotas carry the sign, Exp scales by beta ----
        # xd2[q,p] = q - p ; xdm128[q,p] = q - p - 128  (both input-free)
        xd2 = sb.tile([P, P], f32)
        nc.gpsimd.iota(xd2[:], [[-1, P]], base=0, channel_multiplier=1,
                       allow_small_or_imprecise_dtypes=True)
        xdm2 = sb.tile([P, P], f32)
        nc.gpsimd.affine_select(xdm2[:], xd2[:], [[1, P]], Alu.is_gt, -16384.0,
                                base=0, channel_multiplier=-1)  # keep p - q > 0
        xdm128 = sb.tile([P, P], f32)
        nc.gpsimd.iota(xdm128[:], [[-1, P]], base=-P, channel_multiplier=1,
                       allow_small_or_imprecise_dtypes=True)

        # ldtb[q,p] = exp(-b(p-q)) for q<p else 0; ld2[q,p] = exp(-b(128+p-q))
        ldtb = sb.tile([P, P], bf16)
        nc.scalar.activation(ldtb[:], xdm2[:], Act.Exp, scale=parbeta[:])
        ld2 = sb.tile([P, P], bf16)
        nc.scalar.activation(ld2[:], xdm128[:], Act.Exp, scale=parbeta[:])

        # ---- obs scale: obs_f1 = obs * (adiag * b), bf16 ----------------
        asbb = sb.tile([P, S], bf16)
        nc.vector.tensor_scalar(out=asbb[:], in0=padm[:, 0:S],
                                scalar1=parbeta[:], scalar2=None, op0=Alu.mult)

        # obs_f1x: one zero block, then the J obs blocks -- MM2 reads the
        # same column window as MM1 on this shifted view.
        obs_f1x = sb.tile([P, S + J * S], bf16)
        nc.gpsimd.memset(obs_f1x[:, 0:S], 0.0)
        obs_f1 = obs_f1x[:, S:S + J * S]
        # h0 scaled in two 256-col pieces so the first matmul group can
        # start earlier; h1 whole.
        for lo, hi in ((0, HALF // 2), (HALF // 2, HALF), (HALF, 2 * HALF)):
            nc.vector.tensor_tensor(
                out=obs_f1[:, lo:hi].rearrange("p (j s) -> p j s", s=S),
                in0=obs1[:, lo:hi].rearrange("p (j s) -> p j s", s=S),
                in1=asbb[:].unsqueeze(1).broadcast_to((P, (hi - lo) // S, S)),
                op=Alu.mult)

        # ---- G = MM1 + MM2, fused clip+mu on the PSUM->SBUF pass --------
        g_sb = sb.tile([P, J * S], bf16)
        for h in range(2):
            pch = ps.tile([P, HALF], f32)
            if h == 0:
                # Two fully-paired start/stop groups per 256-col region so
                # the first group begins as soon as its rhs piece is scaled.
                for lo, hi in ((0, HALF // 2), (HALF // 2, HALF)):
                    nc.tensor.matmul(pch[:, lo:hi], lhsT=ldtb[:],
                                     rhs=obs_f1[:, lo:hi],
                                     start=True, stop=False,
                                     skip_group_check=True)
                    nc.tensor.matmul(pch[:, lo:hi], lhsT=ld2[:],
                                     rhs=obs_f1x[:, lo:hi],
                                     start=False, stop=True,
                                     skip_group_check=True)
            else:
                nc.tensor.matmul(pch[:], lhsT=ldtb[:],
                                 rhs=obs_f1[:, h * HALF:(h + 1) * HALF],
                                 start=True, stop=False)
                nc.tensor.matmul(pch[:], lhsT=ld2[:],
                                 rhs=obs_f1x[:, h * HALF:(h + 1) * HALF],
                                 start=False, stop=True,
                                 skip_group_check=True)
            # (gpsimd cannot read PSUM on TRN2 -- both passes on DVE)
            nc.vector.scalar_tensor_tensor(
                out=g_sb[:, h * HALF:(h + 1) * HALF].rearrange(
                    "p (j s) -> p j s", s=S),
                in0=pch[:].rearrange("p (j s) -> p j s", s=S),
                scalar=float(LAM_MIN),
                in1=padm[:, S:2 * S].unsqueeze(1).broadcast_to((P, J // 2, S)),
                op0=Alu.max, op1=Alu.add)
            (nc.sync if h == 0 else nc.scalar).dma_start(
                bass.AP(out_h, h * HALF, [[J * S, P], [1, HALF]]),
                g_sb[:, h * HALF:(h + 1) * HALF])

    nc.compile()
    return nc


_NC_CACHE = None


def _get_nc():
    global _NC_CACHE
    if _NC_CACHE is None:
        _NC_CACHE = build_nc()
    return _NC_CACHE


def _flat_positions(tc_, sc_):
    """Query (t, s) is table cell [p = t mod 128, u = (t div 128)*S + s]
    of the dumped [128, J*S] slice."""
    return (tc_ % P).astype(np.int64) * (J * S) + (tc_ >> 7) * S + sc_


def _make_in_maps(t, s, obs, mu, alpha, beta):
    """Shard by space: core c gets s in [S*c, S*(c+1)).  Returns
    (in_maps, perms) where perms[c] = (flat_out_pos, global_orig_pos)."""
    import ml_dtypes

    t = np.ascontiguousarray(np.asarray(t, dtype=np.int32))
    s = np.ascontiguousarray(np.asarray(s, dtype=np.int32))
    obs = np.ascontiguousarray(np.asarray(obs, dtype=np.int32))
    mu = np.ascontiguousarray(np.asarray(mu, dtype=np.float32))
    alpha = np.asarray(alpha, dtype=np.float32)
    beta = np.ascontiguousarray(np.asarray(beta, dtype=np.float32))
    adiag = np.ascontiguousarray(np.diagonal(alpha)).astype(np.float32)

    parbeta = np.full((P, 1), beta[0], np.float32)

    in_maps, perms = [], []
    for c in range(N_CORES):
        m = (s >> 5) == c
        orig_global = np.nonzero(m)[0]
        flat_pos = _flat_positions(t[m], s[m] & (S - 1))

        o3 = obs[:, S * c:S * (c + 1)].reshape(J, P, S)
        obs1 = np.ascontiguousarray(o3.transpose(1, 0, 2)).reshape(P, J * S)
        padm = np.empty((P, 2 * S), np.float32)
        padm[:, 0:S] = adiag[S * c:S * (c + 1)]
        padm[:, S:2 * S] = mu[S * c:S * (c + 1)]
        in_maps.append({
            "obs1": obs1.astype(np.int8),
            "parbeta": parbeta,
            "padm": padm.astype(ml_dtypes.bfloat16),
        })
        perms.append((flat_pos, orig_global))
    return in_maps, perms


def kernel(t, s, obs, mu, alpha, beta, **_unused):
    nc = _get_nc()
    in_maps, perms = _make_in_maps(t, s, obs, mu, alpha, beta)
    res = run_bass_kernel_spmd(nc, in_maps, core_ids=list(range(N_CORES)))
    out = np.empty(BATCH, np.float32)
    for c in range(N_CORES):
        dev = res.results[c]["out"].reshape(-1)   # [P*J*S]
        out[perms[c][1]] = dev[perms[c][0]]
    return out


if __name__ == "__main__":
    # quick self-check against a numpy re-implementation on random data
    rng = np.random.default_rng(0)
    t = rng.integers(0, N_TIME, BATCH).astype(np.int32)
    s = rng.integers(0, N_SPACE, BATCH).astype(np.int32)
    obs = rng.integers(0, 10, (N_TIME, N_SPACE)).astype(np.int32)
    mu = rng.random(N_SPACE, dtype=np.float32)
    alpha = rng.random((N_SPACE, N_SPACE), dtype=np.float32)
    beta = (rng.random(1, dtype=np.float32) + 0.1).astype(np.float32)

    got = kernel(t=t, s=s, obs=obs, mu=mu, alpha=alpha, beta=beta)

    b = float(beta[0])
    e = np.exp(-b)
    F = np.zeros((N_TIME, N_SPACE), np.float64)
    for tt in range(1, N_TIME):
        F[tt] = e * (F[tt - 1] + obs[tt - 1])
    G = np.clip(mu[None, :] + np.diag(alpha)[None, :] * b * F, LAM_MIN, None)
    want = G[t, s].astype(np.float32)
    err = np.abs(got - want) / np.maximum(np.abs(want), 1e-6)
    print("max rel err:", err.max(), "mean:", err.mean())
